# revision 7
# baseline (speedup 1.0000x reference)
"""CenterLoss on 8 Trainium2 NeuronCores.

Math: the reference builds the full (B, C) squared-distance matrix,
masks it to the one entry (i, labels[i]) per row, clamps AFTER masking
(so the C-1 masked zeros per row each become 1e-12), sums and divides
by B.  Only the gathered center rows matter:

    loss = (sum_i clip(||x_i - c_{l_i}||^2, 1e-12, 1e12)
            + B*(C-1)*1e-12) / B

Sharding: data-parallel over the batch — core k gets rows
[k*256, (k+1)*256) of x/labels and a full replica of centers in DRAM.
Each core gathers its 256 needed center rows with an indirect DMA
(reads 128 KB instead of 51 MB), computes per-row squared distances on
the vector engine, clamps, and writes the 256 distances out.  The host
sums the 8x256 partials and applies the constant clamp correction.
"""

import numpy as np

BATCH = 2048
NUM_CLASSES = 100000
FEAT_DIM = 128
N_CORES = 8
ROWS_PER_CORE = BATCH // N_CORES  # 256
P = 128
TILES_PER_CORE = ROWS_PER_CORE // P  # 2

_CACHE = {}


def _build_bass():
    import concourse.bass as bass
    import concourse.bacc as bacc
    import concourse.mybir as mybir
    from concourse.tile import TileContext

    f32 = mybir.dt.float32
    i32 = mybir.dt.int32

    # Bacc (not raw Bass): its compile passes redistribute semaphore waits
    # that exceed an instruction's sync-wait slots (e.g. the kernel-tail
    # drain), which raw Bass leaves to fail in walrus codegen.
    nc = bacc.Bacc("TRN2", target_bir_lowering=False, debug=False)
    x_d = nc.dram_tensor("x", [ROWS_PER_CORE, FEAT_DIM], f32, kind="ExternalInput")
    lab_d = nc.dram_tensor("labels", [ROWS_PER_CORE, 1], i32, kind="ExternalInput")
    cen_d = nc.dram_tensor(
        "centers", [NUM_CLASSES, FEAT_DIM], f32, kind="ExternalInput"
    )
    out_d = nc.dram_tensor(
        "dists", [TILES_PER_CORE, P], f32, kind="ExternalOutput"
    )

    NT = TILES_PER_CORE
    # Hardware wait-slot limits shape this kernel:
    #  - a TensorTensor encodes ONE sync wait, so both of its operands must
    #    be produced on the DVE (same-sem waits merge into one threshold);
    #  - the kernel-tail Drain encodes ~8 waits, so every extra DMA queue
    #    (one semaphore each) counts — batch all loads/stores into one DMA.
    with TileContext(nc) as tc:
        with tc.tile_pool(name="pool", bufs=2) as pool, tc.tile_pool(
            name="persist", bufs=1
        ) as persist:
            # One DMA per input: x as [128, NT*128], labels as [128, NT]
            x_all = persist.tile([P, NT * FEAT_DIM], f32, tag="x_all")
            nc.sync.dma_start(
                out=x_all[:].rearrange("p (n d) -> p n d", n=NT),
                in_=x_d[:].rearrange("(n p) d -> p n d", p=P),
            )
            idx_all = persist.tile([P, NT], i32, tag="idx_all")
            nc.sync.dma_start(
                out=idx_all[:],
                in_=lab_d[:].rearrange("(n p) o -> p (n o)", p=P),
            )
            # Whole-x DVE copy: downstream TensorTensors read it via the DVE
            # self-semaphore instead of a second DMA semaphore.
            xb = persist.tile([P, NT * FEAT_DIM], f32, tag="xb")
            nc.vector.tensor_copy(out=xb[:], in_=x_all[:])
            s_all = persist.tile([P, NT], f32, tag="s_all")

            for t in range(NT):
                cols = slice(t * FEAT_DIM, (t + 1) * FEAT_DIM)
                c_t = pool.tile([P, FEAT_DIM], f32, tag="c")
                nc.gpsimd.indirect_dma_start(
                    out=c_t[:],
                    out_offset=None,
                    in_=cen_d[:],
                    in_offset=bass.IndirectOffsetOnAxis(
                        ap=idx_all[:, t : t + 1], axis=0
                    ),
                )
                diff = pool.tile([P, FEAT_DIM], f32, tag="diff")
                nc.vector.tensor_copy(out=diff[:], in_=c_t[:])
                nc.vector.tensor_tensor(
                    out=diff[:],
                    in0=xb[:, cols],
                    in1=diff[:],
                    op=mybir.AluOpType.subtract,
                )
                sq = pool.tile([P, FEAT_DIM], f32, tag="sq")
                nc.vector.tensor_tensor(
                    out=sq[:], in0=diff[:], in1=diff[:], op=mybir.AluOpType.mult
                )
                s_t = pool.tile([P, 1], f32, tag="s")
                nc.vector.tensor_reduce(
                    out=s_t[:],
                    in_=sq[:],
                    axis=mybir.AxisListType.X,
                    op=mybir.AluOpType.add,
                )
                # torch clamps after masking: clip(d, 1e-12, 1e12) per row
                nc.vector.tensor_scalar(
                    out=s_all[:, t : t + 1],
                    in0=s_t[:],
                    scalar1=1e-12,
                    scalar2=1e12,
                    op0=mybir.AluOpType.max,
                    op1=mybir.AluOpType.min,
                )
            # One DMA for all outputs: dists[n, p] = s_all[p, n]
            nc.sync.dma_start(
                out=out_d[:].rearrange("n p -> p n"),
                in_=s_all[:],
            )
    nc.compile()
    return nc


def kernel(x, labels, centers):
    from concourse.bass_utils import run_bass_kernel_spmd

    x = np.ascontiguousarray(np.asarray(x, dtype=np.float32))
    centers = np.ascontiguousarray(np.asarray(centers, dtype=np.float32))
    labels = np.ascontiguousarray(
        np.asarray(labels).astype(np.int32).reshape(BATCH, 1)
    )

    if "nc" not in _CACHE:
        _CACHE["nc"] = _build_bass()
    nc = _CACHE["nc"]

    core_ids = list(range(N_CORES))
    in_maps = [
        {
            "x": x[k * ROWS_PER_CORE : (k + 1) * ROWS_PER_CORE],
            "labels": labels[k * ROWS_PER_CORE : (k + 1) * ROWS_PER_CORE],
            "centers": centers,
        }
        for k in core_ids
    ]

    res = run_bass_kernel_spmd(nc, in_maps, core_ids)
    _CACHE["last_results"] = res

    dists = np.concatenate([res.results[k]["dists"].reshape(-1) for k in core_ids])
    # B*(C-1) masked zeros, each clamped up to 1e-12 by the reference.
    total = dists.sum(dtype=np.float64) + BATCH * (NUM_CLASSES - 1) * 1e-12
    return np.float32(total / BATCH)


# revision 17
# speedup vs baseline: 1.2561x; 1.2561x over previous
"""CenterLoss on 8 Trainium2 NeuronCores.

Math: the reference builds the full (B, C) squared-distance matrix,
masks it to the one entry (i, labels[i]) per row, clamps AFTER masking
(so the C-1 masked zeros per row each become 1e-12), sums and divides
by B.  Only the gathered center rows matter:

    loss = (sum_i clip(||x_i - c_{l_i}||^2, 1e-12, 1e12)
            + B*(C-1)*1e-12) / B

Sharding: data-parallel over the batch — core k gets rows
[k*256, (k+1)*256) of x/labels and a full replica of centers in DRAM.
Each core gathers its 256 needed center rows with an indirect DMA
(reads 128 KB instead of 51 MB), computes per-row squared distances on
the vector engine, clamps, and writes the 256 distances out.  The host
sums the 8x256 partials and applies the constant clamp correction.
"""

import os

import numpy as np

BATCH = 2048
NUM_CLASSES = 100000
FEAT_DIM = 128
N_CORES = 8
ROWS_PER_CORE = BATCH // N_CORES  # 256
P = 128
TILES_PER_CORE = ROWS_PER_CORE // P  # 2

_CACHE = {}


def _build_raw():
    """Hand-synchronized raw-Bass kernel (no TileContext).

    Tile's entry barrier + exit drain/double-barrier/sem-clear cost
    ~10-13us of fixed overhead on a ~7us body. With manual semaphores the
    kernel is: labels DMA -> 2 indirect gathers (gpsimd), x DMA in
    parallel, a DVE chain (sub/sq/row-reduce/clamp) where tile 0's
    compute overlaps tile 1's gather, and one output DMA. Semaphores are
    cleared at the end so re-executing the same loaded NEFF stays correct.
    """
    from contextlib import ExitStack

    import concourse.bass as bass
    import concourse.mybir as mybir

    f32 = mybir.dt.float32
    i32 = mybir.dt.int32
    NT = TILES_PER_CORE
    D = FEAT_DIM

    # Row i of this core's shard maps to (partition, tile) = (i // NT,
    # i % NT): with row-index = p*NT + n every DMA's innermost dim is
    # contiguous in DRAM (tile-major row = n*P + p would stride it).
    nc = bass.Bass()
    x_d = nc.dram_tensor("x", [ROWS_PER_CORE, D], f32, kind="ExternalInput")
    lab_d = nc.dram_tensor("labels", [ROWS_PER_CORE, 1], i32, kind="ExternalInput")
    cen_d = nc.dram_tensor("centers", [NUM_CLASSES, D], f32, kind="ExternalInput")
    out_d = nc.dram_tensor("dists", [ROWS_PER_CORE, 1], f32, kind="ExternalOutput")

    with ExitStack() as ctx:
        x_all = ctx.enter_context(nc.sbuf_tensor([P, NT * D], f32))
        idx = ctx.enter_context(nc.sbuf_tensor([P, NT], i32))
        c_all = ctx.enter_context(nc.sbuf_tensor([P, NT * D], f32))
        dif = ctx.enter_context(nc.sbuf_tensor([P, NT * D], f32))
        sq = ctx.enter_context(nc.sbuf_tensor([P, NT * D], f32))
        s_all = ctx.enter_context(nc.sbuf_tensor([P, NT], f32))
        s_lab = ctx.enter_context(nc.semaphore("s_lab"))
        s_x = ctx.enter_context(nc.semaphore("s_x"))
        s_g = ctx.enter_context(nc.semaphore("s_g"))
        s_v = ctx.enter_context(nc.semaphore("s_v"))
        s_out = ctx.enter_context(nc.semaphore("s_out"))
        s_d = ctx.enter_context(nc.semaphore("s_d"))
        block = ctx.enter_context(nc.Block(no_gpsimd_drain=True))

        @block.sync
        def _(sync):
            sync.dma_start(
                out=idx[:], in_=lab_d[:].rearrange("(p n) o -> p (n o)", n=NT)
            ).then_inc(s_lab, 16)
            sync.dma_start(
                out=x_all[:].rearrange("p (n d) -> p n d", n=NT),
                in_=x_d[:].rearrange("(p n) d -> p n d", n=NT),
            ).then_inc(s_x, 16)
            sync.wait_ge(s_v, 1)
            sync.dma_start(
                out=out_d[:].rearrange("(p n) o -> p (n o)", n=NT), in_=s_all[:]
            ).then_inc(s_out, 16)
            # All sem incs have landed (proven by the waits that gated each
            # consumer), so clear for the next execution of this NEFF.
            sync.wait_ge(s_out, 16)
            for s in (s_lab, s_x, s_g, s_v, s_out, s_d):
                sync.sem_clear(s)

        @block.gpsimd
        def _(gpsimd):
            gpsimd.wait_ge(s_lab, 16)
            for t in range(NT):
                gpsimd.indirect_dma_start(
                    out=c_all[:, t * D : (t + 1) * D],
                    out_offset=None,
                    in_=cen_d[:],
                    in_offset=bass.IndirectOffsetOnAxis(
                        ap=idx[:, t : t + 1], axis=0
                    ),
                ).then_inc(s_g, 16)

        @block.vector
        def _(vector):
            # DVE RAW hazards between back-to-back ops are real (the pipe
            # flush only covers output hazards), so dependent ops chain
            # through the s_d self-semaphore exactly like Tile emits.
            ticks = 0
            vector.wait_ge(s_x, 16)
            for t in range(NT):
                cols = slice(t * D, (t + 1) * D)
                vector.wait_ge(s_g, 16 * (t + 1))
                vector.tensor_tensor(
                    out=dif[:, cols],
                    in0=x_all[:, cols],
                    in1=c_all[:, cols],
                    op=mybir.AluOpType.subtract,
                ).then_inc(s_d, 1)
                ticks += 1
                vector.wait_ge(s_d, ticks)
                vector.tensor_tensor(
                    out=sq[:, cols],
                    in0=dif[:, cols],
                    in1=dif[:, cols],
                    op=mybir.AluOpType.mult,
                ).then_inc(s_d, 1)
                ticks += 1
                vector.wait_ge(s_d, ticks)
                vector.tensor_reduce(
                    out=s_all[:, t : t + 1],
                    in_=sq[:].rearrange("p (n d) -> p n d", n=NT)[:, t : t + 1, :],
                    axis=mybir.AxisListType.X,
                    op=mybir.AluOpType.add,
                ).then_inc(s_d, 1)
                ticks += 1
            vector.wait_ge(s_d, ticks)
            # torch clamps after masking: clip(d, 1e-12, 1e12) per row
            vector.tensor_scalar(
                out=s_all[:],
                in0=s_all[:],
                scalar1=1e-12,
                scalar2=1e12,
                op0=mybir.AluOpType.max,
                op1=mybir.AluOpType.min,
            ).then_inc(s_v, 1)

    return nc


def _build_bass():
    import concourse.bass as bass
    import concourse.bacc as bacc
    import concourse.mybir as mybir
    from concourse.tile import TileContext

    f32 = mybir.dt.float32
    i32 = mybir.dt.int32

    # Bacc (not raw Bass): its compile passes redistribute semaphore waits
    # that exceed an instruction's sync-wait slots (e.g. the kernel-tail
    # drain), which raw Bass leaves to fail in walrus codegen.
    nc = bacc.Bacc("TRN2", target_bir_lowering=False, debug=False)
    x_d = nc.dram_tensor("x", [ROWS_PER_CORE, FEAT_DIM], f32, kind="ExternalInput")
    lab_d = nc.dram_tensor("labels", [ROWS_PER_CORE, 1], i32, kind="ExternalInput")
    cen_d = nc.dram_tensor(
        "centers", [NUM_CLASSES, FEAT_DIM], f32, kind="ExternalInput"
    )
    out_d = nc.dram_tensor(
        "dists", [TILES_PER_CORE, P], f32, kind="ExternalOutput"
    )

    NT = TILES_PER_CORE
    # Hardware wait-slot limits shape this kernel:
    #  - a TensorTensor encodes ONE sync wait, so both of its operands must
    #    be produced on the DVE (same-sem waits merge into one threshold);
    #  - the kernel-tail Drain encodes ~8 waits, so every extra DMA queue
    #    (one semaphore each) counts — batch all loads/stores into one DMA.
    with TileContext(nc) as tc:
        with tc.tile_pool(name="pool", bufs=2) as pool, tc.tile_pool(
            name="persist", bufs=1
        ) as persist:
            # One DMA per input: x as [128, NT*128], labels as [128, NT]
            x_all = persist.tile([P, NT * FEAT_DIM], f32, tag="x_all")
            nc.sync.dma_start(
                out=x_all[:].rearrange("p (n d) -> p n d", n=NT),
                in_=x_d[:].rearrange("(n p) d -> p n d", p=P),
            )
            idx_all = persist.tile([P, NT], i32, tag="idx_all")
            nc.sync.dma_start(
                out=idx_all[:],
                in_=lab_d[:].rearrange("(n p) o -> p (n o)", p=P),
            )
            # Whole-x DVE copy: downstream TensorTensors read it via the DVE
            # self-semaphore instead of a second DMA semaphore.
            xb = persist.tile([P, NT * FEAT_DIM], f32, tag="xb")
            nc.vector.tensor_copy(out=xb[:], in_=x_all[:])
            s_all = persist.tile([P, NT], f32, tag="s_all")

            for t in range(NT):
                cols = slice(t * FEAT_DIM, (t + 1) * FEAT_DIM)
                c_t = pool.tile([P, FEAT_DIM], f32, tag="c")
                nc.gpsimd.indirect_dma_start(
                    out=c_t[:],
                    out_offset=None,
                    in_=cen_d[:],
                    in_offset=bass.IndirectOffsetOnAxis(
                        ap=idx_all[:, t : t + 1], axis=0
                    ),
                )
                diff = pool.tile([P, FEAT_DIM], f32, tag="diff")
                nc.vector.tensor_copy(out=diff[:], in_=c_t[:])
                nc.vector.tensor_tensor(
                    out=diff[:],
                    in0=xb[:, cols],
                    in1=diff[:],
                    op=mybir.AluOpType.subtract,
                )
                sq = pool.tile([P, FEAT_DIM], f32, tag="sq")
                nc.vector.tensor_tensor(
                    out=sq[:], in0=diff[:], in1=diff[:], op=mybir.AluOpType.mult
                )
                s_t = pool.tile([P, 1], f32, tag="s")
                nc.vector.tensor_reduce(
                    out=s_t[:],
                    in_=sq[:],
                    axis=mybir.AxisListType.X,
                    op=mybir.AluOpType.add,
                )
                # torch clamps after masking: clip(d, 1e-12, 1e12) per row
                nc.vector.tensor_scalar(
                    out=s_all[:, t : t + 1],
                    in0=s_t[:],
                    scalar1=1e-12,
                    scalar2=1e12,
                    op0=mybir.AluOpType.max,
                    op1=mybir.AluOpType.min,
                )
            # One DMA for all outputs: dists[n, p] = s_all[p, n]
            nc.sync.dma_start(
                out=out_d[:].rearrange("n p -> p n"),
                in_=s_all[:],
            )
    nc.compile()
    return nc


def kernel(x, labels, centers):
    from concourse.bass_utils import run_bass_kernel_spmd

    x = np.ascontiguousarray(np.asarray(x, dtype=np.float32))
    centers = np.ascontiguousarray(np.asarray(centers, dtype=np.float32))
    labels = np.ascontiguousarray(
        np.asarray(labels).astype(np.int32).reshape(BATCH, 1)
    )

    impl = os.environ.get("CENTERLOSS_IMPL", "raw")
    if ("nc", impl) not in _CACHE:
        _CACHE[("nc", impl)] = _build_raw() if impl == "raw" else _build_bass()
    nc = _CACHE[("nc", impl)]

    core_ids = list(range(N_CORES))
    in_maps = [
        {
            "x": x[k * ROWS_PER_CORE : (k + 1) * ROWS_PER_CORE],
            "labels": labels[k * ROWS_PER_CORE : (k + 1) * ROWS_PER_CORE],
            "centers": centers,
        }
        for k in core_ids
    ]

    res = run_bass_kernel_spmd(nc, in_maps, core_ids)
    _CACHE["last_results"] = res

    dists = np.concatenate([res.results[k]["dists"].reshape(-1) for k in core_ids])
    # B*(C-1) masked zeros, each clamped up to 1e-12 by the reference.
    total = dists.sum(dtype=np.float64) + BATCH * (NUM_CLASSES - 1) * 1e-12
    return np.float32(total / BATCH)


# revision 20
# speedup vs baseline: 1.2850x; 1.0230x over previous
"""CenterLoss on 8 Trainium2 NeuronCores.

Math: the reference builds the full (B, C) squared-distance matrix,
masks it to the one entry (i, labels[i]) per row, clamps AFTER masking
(so the C-1 masked zeros per row each become 1e-12), sums and divides
by B.  Only the gathered center rows matter:

    loss = (sum_i clip(||x_i - c_{l_i}||^2, 1e-12, 1e12)
            + B*(C-1)*1e-12) / B

Sharding: data-parallel over the batch — core k gets rows
[k*256, (k+1)*256) of x/labels and a full replica of centers in DRAM.
Each core gathers its 256 needed center rows with an indirect DMA
(reads 128 KB instead of 51 MB), computes per-row squared distances on
the vector engine, clamps, and writes the 256 distances out.  The host
sums the 8x256 partials and applies the constant clamp correction.
"""

import os

import numpy as np

BATCH = 2048
NUM_CLASSES = 100000
FEAT_DIM = 128
N_CORES = 8
ROWS_PER_CORE = BATCH // N_CORES  # 256
P = 128
TILES_PER_CORE = ROWS_PER_CORE // P  # 2

_CACHE = {}


def _build_raw():
    """Hand-synchronized raw-Bass kernel (no TileContext).

    Tile's entry barrier + exit drain/double-barrier/sem-clear cost
    ~10-13us of fixed overhead on a ~7us body. With manual semaphores the
    kernel is: labels DMA -> 2 indirect gathers (gpsimd), x DMA in
    parallel, a DVE chain (sub/sq/row-reduce/clamp) where tile 0's
    compute overlaps tile 1's gather, and one output DMA. Semaphores are
    cleared at the end so re-executing the same loaded NEFF stays correct.
    """
    from contextlib import ExitStack

    import concourse.bass as bass
    import concourse.mybir as mybir

    f32 = mybir.dt.float32
    i32 = mybir.dt.int32
    NT = TILES_PER_CORE
    D = FEAT_DIM

    # Row i of this core's shard maps to (partition, tile) = (i // NT,
    # i % NT): with row-index = p*NT + n every DMA's innermost dim is
    # contiguous in DRAM (tile-major row = n*P + p would stride it).
    nc = bass.Bass()
    x_d = nc.dram_tensor("x", [ROWS_PER_CORE, D], f32, kind="ExternalInput")
    lab_d = nc.dram_tensor("labels", [ROWS_PER_CORE, 1], i32, kind="ExternalInput")
    cen_d = nc.dram_tensor("centers", [NUM_CLASSES, D], f32, kind="ExternalInput")
    out_d = nc.dram_tensor("dists", [ROWS_PER_CORE, 1], f32, kind="ExternalOutput")

    with ExitStack() as ctx:
        x_all = ctx.enter_context(nc.sbuf_tensor([P, NT * D], f32))
        idx = ctx.enter_context(nc.sbuf_tensor([P, NT], i32))
        c_all = ctx.enter_context(nc.sbuf_tensor([P, NT * D], f32))
        dif = ctx.enter_context(nc.sbuf_tensor([P, NT * D], f32))
        sq = ctx.enter_context(nc.sbuf_tensor([P, NT * D], f32))
        s_all = ctx.enter_context(nc.sbuf_tensor([P, NT], f32))
        s_lab = ctx.enter_context(nc.semaphore("s_lab"))
        s_x = ctx.enter_context(nc.semaphore("s_x"))
        s_g = ctx.enter_context(nc.semaphore("s_g"))
        s_v = ctx.enter_context(nc.semaphore("s_v"))
        s_out = ctx.enter_context(nc.semaphore("s_out"))
        s_d = ctx.enter_context(nc.semaphore("s_d"))

        # Semaphores are NOT guaranteed zero at NEFF load (a prior kernel
        # or interrupted execution can leave residue, which makes waits
        # pass early and silently corrupts rows). Clear them, then sync
        # all engines with the NRT pseudo barrier (runtime-expanded, so it
        # does not itself depend on bass sems) — the same pattern Bass's
        # lowering preamble uses.
        for s in (s_lab, s_x, s_g, s_v, s_out, s_d):
            nc.gpsimd.sem_clear(s)
        nc._nrt_pseudo_barrier()

        block = ctx.enter_context(nc.Block(no_gpsimd_drain=True))

        @block.sync
        def _(sync):
            sync.dma_start(
                out=idx[:], in_=lab_d[:].rearrange("(p n) o -> p (n o)", n=NT)
            ).then_inc(s_lab, 16)
            sync.dma_start(
                out=x_all[:].rearrange("p (n d) -> p n d", n=NT),
                in_=x_d[:].rearrange("(p n) d -> p n d", n=NT),
            ).then_inc(s_x, 16)
            sync.wait_ge(s_v, 1)
            sync.dma_start(
                out=out_d[:].rearrange("(p n) o -> p (n o)", n=NT), in_=s_all[:]
            ).then_inc(s_out, 16)
            # No exit clears needed: the entry clears make every execution
            # self-correcting, and the block-exit SP drain waits for the
            # output DMA before the engines halt.

        @block.gpsimd
        def _(gpsimd):
            gpsimd.wait_ge(s_lab, 16)
            for t in range(NT):
                gpsimd.indirect_dma_start(
                    out=c_all[:, t * D : (t + 1) * D],
                    out_offset=None,
                    in_=cen_d[:],
                    in_offset=bass.IndirectOffsetOnAxis(
                        ap=idx[:, t : t + 1], axis=0
                    ),
                ).then_inc(s_g, 16)

        @block.vector
        def _(vector):
            # DVE RAW hazards between back-to-back ops are real (the pipe
            # flush only covers output hazards), so dependent ops chain
            # through the s_d self-semaphore exactly like Tile emits.
            ticks = 0
            vector.wait_ge(s_x, 16)
            for t in range(NT):
                cols = slice(t * D, (t + 1) * D)
                vector.wait_ge(s_g, 16 * (t + 1))
                vector.tensor_tensor(
                    out=dif[:, cols],
                    in0=x_all[:, cols],
                    in1=c_all[:, cols],
                    op=mybir.AluOpType.subtract,
                ).then_inc(s_d, 1)
                ticks += 1
                vector.wait_ge(s_d, ticks)
                vector.tensor_tensor(
                    out=sq[:, cols],
                    in0=dif[:, cols],
                    in1=dif[:, cols],
                    op=mybir.AluOpType.mult,
                ).then_inc(s_d, 1)
                ticks += 1
                vector.wait_ge(s_d, ticks)
                vector.tensor_reduce(
                    out=s_all[:, t : t + 1],
                    in_=sq[:].rearrange("p (n d) -> p n d", n=NT)[:, t : t + 1, :],
                    axis=mybir.AxisListType.X,
                    op=mybir.AluOpType.add,
                ).then_inc(s_d, 1)
                ticks += 1
            vector.wait_ge(s_d, ticks)
            # torch clamps after masking: clip(d, 1e-12, 1e12) per row
            vector.tensor_scalar(
                out=s_all[:],
                in0=s_all[:],
                scalar1=1e-12,
                scalar2=1e12,
                op0=mybir.AluOpType.max,
                op1=mybir.AluOpType.min,
            ).then_inc(s_v, 1)

    return nc


def _build_bass():
    import concourse.bass as bass
    import concourse.bacc as bacc
    import concourse.mybir as mybir
    from concourse.tile import TileContext

    f32 = mybir.dt.float32
    i32 = mybir.dt.int32

    # Bacc (not raw Bass): its compile passes redistribute semaphore waits
    # that exceed an instruction's sync-wait slots (e.g. the kernel-tail
    # drain), which raw Bass leaves to fail in walrus codegen.
    nc = bacc.Bacc("TRN2", target_bir_lowering=False, debug=False)
    x_d = nc.dram_tensor("x", [ROWS_PER_CORE, FEAT_DIM], f32, kind="ExternalInput")
    lab_d = nc.dram_tensor("labels", [ROWS_PER_CORE, 1], i32, kind="ExternalInput")
    cen_d = nc.dram_tensor(
        "centers", [NUM_CLASSES, FEAT_DIM], f32, kind="ExternalInput"
    )
    out_d = nc.dram_tensor(
        "dists", [TILES_PER_CORE, P], f32, kind="ExternalOutput"
    )

    NT = TILES_PER_CORE
    # Hardware wait-slot limits shape this kernel:
    #  - a TensorTensor encodes ONE sync wait, so both of its operands must
    #    be produced on the DVE (same-sem waits merge into one threshold);
    #  - the kernel-tail Drain encodes ~8 waits, so every extra DMA queue
    #    (one semaphore each) counts — batch all loads/stores into one DMA.
    with TileContext(nc) as tc:
        with tc.tile_pool(name="pool", bufs=2) as pool, tc.tile_pool(
            name="persist", bufs=1
        ) as persist:
            # One DMA per input: x as [128, NT*128], labels as [128, NT]
            x_all = persist.tile([P, NT * FEAT_DIM], f32, tag="x_all")
            nc.sync.dma_start(
                out=x_all[:].rearrange("p (n d) -> p n d", n=NT),
                in_=x_d[:].rearrange("(n p) d -> p n d", p=P),
            )
            idx_all = persist.tile([P, NT], i32, tag="idx_all")
            nc.sync.dma_start(
                out=idx_all[:],
                in_=lab_d[:].rearrange("(n p) o -> p (n o)", p=P),
            )
            # Whole-x DVE copy: downstream TensorTensors read it via the DVE
            # self-semaphore instead of a second DMA semaphore.
            xb = persist.tile([P, NT * FEAT_DIM], f32, tag="xb")
            nc.vector.tensor_copy(out=xb[:], in_=x_all[:])
            s_all = persist.tile([P, NT], f32, tag="s_all")

            for t in range(NT):
                cols = slice(t * FEAT_DIM, (t + 1) * FEAT_DIM)
                c_t = pool.tile([P, FEAT_DIM], f32, tag="c")
                nc.gpsimd.indirect_dma_start(
                    out=c_t[:],
                    out_offset=None,
                    in_=cen_d[:],
                    in_offset=bass.IndirectOffsetOnAxis(
                        ap=idx_all[:, t : t + 1], axis=0
                    ),
                )
                diff = pool.tile([P, FEAT_DIM], f32, tag="diff")
                nc.vector.tensor_copy(out=diff[:], in_=c_t[:])
                nc.vector.tensor_tensor(
                    out=diff[:],
                    in0=xb[:, cols],
                    in1=diff[:],
                    op=mybir.AluOpType.subtract,
                )
                sq = pool.tile([P, FEAT_DIM], f32, tag="sq")
                nc.vector.tensor_tensor(
                    out=sq[:], in0=diff[:], in1=diff[:], op=mybir.AluOpType.mult
                )
                s_t = pool.tile([P, 1], f32, tag="s")
                nc.vector.tensor_reduce(
                    out=s_t[:],
                    in_=sq[:],
                    axis=mybir.AxisListType.X,
                    op=mybir.AluOpType.add,
                )
                # torch clamps after masking: clip(d, 1e-12, 1e12) per row
                nc.vector.tensor_scalar(
                    out=s_all[:, t : t + 1],
                    in0=s_t[:],
                    scalar1=1e-12,
                    scalar2=1e12,
                    op0=mybir.AluOpType.max,
                    op1=mybir.AluOpType.min,
                )
            # One DMA for all outputs: dists[n, p] = s_all[p, n]
            nc.sync.dma_start(
                out=out_d[:].rearrange("n p -> p n"),
                in_=s_all[:],
            )
    nc.compile()
    return nc


def kernel(x, labels, centers):
    from concourse.bass_utils import run_bass_kernel_spmd

    x = np.ascontiguousarray(np.asarray(x, dtype=np.float32))
    centers = np.ascontiguousarray(np.asarray(centers, dtype=np.float32))
    labels = np.ascontiguousarray(
        np.asarray(labels).astype(np.int32).reshape(BATCH, 1)
    )

    impl = os.environ.get("CENTERLOSS_IMPL", "raw")
    if ("nc", impl) not in _CACHE:
        _CACHE[("nc", impl)] = _build_raw() if impl == "raw" else _build_bass()
    nc = _CACHE[("nc", impl)]

    core_ids = list(range(N_CORES))
    in_maps = [
        {
            "x": x[k * ROWS_PER_CORE : (k + 1) * ROWS_PER_CORE],
            "labels": labels[k * ROWS_PER_CORE : (k + 1) * ROWS_PER_CORE],
            "centers": centers,
        }
        for k in core_ids
    ]

    res = run_bass_kernel_spmd(nc, in_maps, core_ids)
    _CACHE["last_results"] = res

    dists = np.concatenate([res.results[k]["dists"].reshape(-1) for k in core_ids])
    # B*(C-1) masked zeros, each clamped up to 1e-12 by the reference.
    total = dists.sum(dtype=np.float64) + BATCH * (NUM_CLASSES - 1) * 1e-12
    return np.float32(total / BATCH)


# revision 30
# speedup vs baseline: 1.5461x; 1.2032x over previous
"""CenterLoss on 8 Trainium2 NeuronCores.

Math: the reference builds the full (B, C) squared-distance matrix,
masks it to the one entry (i, labels[i]) per row, clamps AFTER masking
(so the C-1 masked zeros per row each become 1e-12), sums and divides
by B.  Only the gathered center rows matter:

    loss = (sum_i clip(||x_i - c_{l_i}||^2, 1e-12, 1e12)
            + B*(C-1)*1e-12) / B

Sharding: data-parallel over the batch — core k gets rows
[k*256, (k+1)*256) of x/labels and a full replica of centers in DRAM.
Each core gathers its 256 needed center rows with an indirect DMA
(reads 128 KB instead of 51 MB), computes per-row squared distances on
the vector engine, clamps, and writes the 256 distances out.  The host
sums the 8x256 partials and applies the constant clamp correction.
"""

import os

import numpy as np

BATCH = 2048
NUM_CLASSES = 100000
FEAT_DIM = 128
N_CORES = 8
ROWS_PER_CORE = BATCH // N_CORES  # 256
P = 128
TILES_PER_CORE = ROWS_PER_CORE // P  # 2

_CACHE = {}


def _build_raw():
    """Hand-synchronized raw-Bass kernel (no TileContext).

    Tile's entry barrier + exit drain/double-barrier/sem-clear cost
    ~10-13us of fixed overhead on a ~7us body. With manual semaphores the
    kernel is: labels DMA -> 2 indirect gathers (gpsimd), x DMA in
    parallel, a DVE chain (sub/sq/row-reduce/clamp) where tile 0's
    compute overlaps tile 1's gather, and one output DMA. Semaphores are
    cleared at the end so re-executing the same loaded NEFF stays correct.
    """
    from contextlib import ExitStack

    import concourse.bass as bass
    import concourse.mybir as mybir

    f32 = mybir.dt.float32
    i32 = mybir.dt.int32
    NT = TILES_PER_CORE
    D = FEAT_DIM

    # Row i of this core's shard maps to (partition, tile) = (i // NT,
    # i % NT): with row-index = p*NT + n every DMA's innermost dim is
    # contiguous in DRAM (tile-major row = n*P + p would stride it).
    nc = bass.Bass()
    x_d = nc.dram_tensor("x", [ROWS_PER_CORE, D], f32, kind="ExternalInput")
    lab_d = nc.dram_tensor("labels", [ROWS_PER_CORE, 1], i32, kind="ExternalInput")
    cen_d = nc.dram_tensor("centers", [NUM_CLASSES, D], f32, kind="ExternalInput")
    out_d = nc.dram_tensor("dists", [ROWS_PER_CORE, 1], f32, kind="ExternalOutput")

    with ExitStack() as ctx:
        x_all = ctx.enter_context(nc.sbuf_tensor([P, NT * D], f32))
        idx = ctx.enter_context(nc.sbuf_tensor([P, NT], i32))
        c_all = ctx.enter_context(nc.sbuf_tensor([P, NT * D], f32))
        dif = ctx.enter_context(nc.sbuf_tensor([P, NT * D], f32))
        sq = ctx.enter_context(nc.sbuf_tensor([P, NT * D], f32))
        s_all = ctx.enter_context(nc.sbuf_tensor([P, NT], f32))
        s_lab = ctx.enter_context(nc.semaphore("s_lab"))
        s_x = ctx.enter_context(nc.semaphore("s_x"))
        s_g = ctx.enter_context(nc.semaphore("s_g"))
        s_v = ctx.enter_context(nc.semaphore("s_v"))
        s_out = ctx.enter_context(nc.semaphore("s_out"))
        s_d = ctx.enter_context(nc.semaphore("s_d"))

        # Semaphores are NOT guaranteed zero at NEFF load (a prior kernel
        # or interrupted execution can leave residue, which makes waits
        # pass early and silently corrupts rows). Clear them, then sync
        # all engines with the NRT pseudo barrier (runtime-expanded, so it
        # does not itself depend on bass sems) — the same pattern Bass's
        # lowering preamble uses.
        for s in (s_x, s_g, s_v, s_out, s_d):
            nc.gpsimd.sem_clear(s)
        # The labels load is the long pole (DMA + ~1.5us completion-sem
        # latency gate the gathers), so issue it BEFORE the barrier: sync
        # clears s_lab itself (same-engine order makes clear-then-inc
        # race-free) and the consumer's wait sits behind the barrier.
        nc.sync.sem_clear(s_lab)
        nc.sync.dma_start(
            out=idx[:], in_=lab_d[:].rearrange("(p n) o -> p (n o)", n=NT)
        ).then_inc(s_lab, 16)
        nc._nrt_pseudo_barrier()

        block = ctx.enter_context(nc.Block(no_gpsimd_drain=True))

        @block.sync
        def _(sync):
            sync.dma_start(
                out=x_all[:].rearrange("p (n d) -> p n d", n=NT),
                in_=x_d[:].rearrange("(p n) d -> p n d", n=NT),
            ).then_inc(s_x, 16)
            sync.wait_ge(s_v, 1)
            sync.dma_start(
                out=out_d[:].rearrange("(p n) o -> p (n o)", n=NT), in_=s_all[:]
            ).then_inc(s_out, 16)
            # No exit clears needed: the entry clears make every execution
            # self-correcting, and the block-exit SP drain waits for the
            # output DMA before the engines halt.

        @block.gpsimd
        def _(gpsimd):
            # Two gathers of 128 rows, not one of 256: consecutive
            # indirect DMAs round-robin onto different SWDGE queues, so
            # their per-descriptor payload processing (~23ns/row/queue)
            # overlaps. A single 256-row gather serializes all payload on
            # one queue (+6.5us measured). Offset tables must be SBUF.
            gpsimd.wait_ge(s_lab, 16)
            for t in range(NT):
                gpsimd.indirect_dma_start(
                    out=c_all[:, t * D : (t + 1) * D],
                    out_offset=None,
                    in_=cen_d[:],
                    in_offset=bass.IndirectOffsetOnAxis(
                        ap=idx[:, t : t + 1], axis=0
                    ),
                ).then_inc(s_g, 16)

        @block.vector
        def _(vector):
            # DVE RAW hazards between back-to-back ops are real (the pipe
            # flush only covers output hazards), so dependent ops chain
            # through the s_d self-semaphore exactly like Tile emits.
            # Batched whole-width ops (one sub/mul/reduce over both tiles)
            # halve the per-element DVE cost vs per-tile ops. The torch
            # clamp clip(d, 1e-12, 1e12) is applied on the host: d here is
            # a direct sum of squares (>= 0, and ~144..384 for this data),
            # so the device-side clamp can never bind.
            vector.wait_ge(s_x, 16)
            vector.wait_ge(s_g, 16 * NT)
            vector.tensor_tensor(
                out=dif[:],
                in0=x_all[:],
                in1=c_all[:],
                op=mybir.AluOpType.subtract,
            ).then_inc(s_d, 1)
            vector.wait_ge(s_d, 1)
            vector.tensor_tensor(
                out=sq[:], in0=dif[:], in1=dif[:], op=mybir.AluOpType.mult
            ).then_inc(s_d, 1)
            vector.wait_ge(s_d, 2)
            vector.tensor_reduce(
                out=s_all[:],
                in_=sq[:].rearrange("p (n d) -> p n d", n=NT),
                axis=mybir.AxisListType.X,
                op=mybir.AluOpType.add,
            ).then_inc(s_v, 1)

    return nc


def _build_bass():
    import concourse.bass as bass
    import concourse.bacc as bacc
    import concourse.mybir as mybir
    from concourse.tile import TileContext

    f32 = mybir.dt.float32
    i32 = mybir.dt.int32

    # Bacc (not raw Bass): its compile passes redistribute semaphore waits
    # that exceed an instruction's sync-wait slots (e.g. the kernel-tail
    # drain), which raw Bass leaves to fail in walrus codegen.
    nc = bacc.Bacc("TRN2", target_bir_lowering=False, debug=False)
    x_d = nc.dram_tensor("x", [ROWS_PER_CORE, FEAT_DIM], f32, kind="ExternalInput")
    lab_d = nc.dram_tensor("labels", [ROWS_PER_CORE, 1], i32, kind="ExternalInput")
    cen_d = nc.dram_tensor(
        "centers", [NUM_CLASSES, FEAT_DIM], f32, kind="ExternalInput"
    )
    out_d = nc.dram_tensor(
        "dists", [TILES_PER_CORE, P], f32, kind="ExternalOutput"
    )

    NT = TILES_PER_CORE
    # Hardware wait-slot limits shape this kernel:
    #  - a TensorTensor encodes ONE sync wait, so both of its operands must
    #    be produced on the DVE (same-sem waits merge into one threshold);
    #  - the kernel-tail Drain encodes ~8 waits, so every extra DMA queue
    #    (one semaphore each) counts — batch all loads/stores into one DMA.
    with TileContext(nc) as tc:
        with tc.tile_pool(name="pool", bufs=2) as pool, tc.tile_pool(
            name="persist", bufs=1
        ) as persist:
            # One DMA per input: x as [128, NT*128], labels as [128, NT]
            x_all = persist.tile([P, NT * FEAT_DIM], f32, tag="x_all")
            nc.sync.dma_start(
                out=x_all[:].rearrange("p (n d) -> p n d", n=NT),
                in_=x_d[:].rearrange("(n p) d -> p n d", p=P),
            )
            idx_all = persist.tile([P, NT], i32, tag="idx_all")
            nc.sync.dma_start(
                out=idx_all[:],
                in_=lab_d[:].rearrange("(n p) o -> p (n o)", p=P),
            )
            # Whole-x DVE copy: downstream TensorTensors read it via the DVE
            # self-semaphore instead of a second DMA semaphore.
            xb = persist.tile([P, NT * FEAT_DIM], f32, tag="xb")
            nc.vector.tensor_copy(out=xb[:], in_=x_all[:])
            s_all = persist.tile([P, NT], f32, tag="s_all")

            for t in range(NT):
                cols = slice(t * FEAT_DIM, (t + 1) * FEAT_DIM)
                c_t = pool.tile([P, FEAT_DIM], f32, tag="c")
                nc.gpsimd.indirect_dma_start(
                    out=c_t[:],
                    out_offset=None,
                    in_=cen_d[:],
                    in_offset=bass.IndirectOffsetOnAxis(
                        ap=idx_all[:, t : t + 1], axis=0
                    ),
                )
                diff = pool.tile([P, FEAT_DIM], f32, tag="diff")
                nc.vector.tensor_copy(out=diff[:], in_=c_t[:])
                nc.vector.tensor_tensor(
                    out=diff[:],
                    in0=xb[:, cols],
                    in1=diff[:],
                    op=mybir.AluOpType.subtract,
                )
                sq = pool.tile([P, FEAT_DIM], f32, tag="sq")
                nc.vector.tensor_tensor(
                    out=sq[:], in0=diff[:], in1=diff[:], op=mybir.AluOpType.mult
                )
                s_t = pool.tile([P, 1], f32, tag="s")
                nc.vector.tensor_reduce(
                    out=s_t[:],
                    in_=sq[:],
                    axis=mybir.AxisListType.X,
                    op=mybir.AluOpType.add,
                )
                # torch clamps after masking: clip(d, 1e-12, 1e12) per row
                nc.vector.tensor_scalar(
                    out=s_all[:, t : t + 1],
                    in0=s_t[:],
                    scalar1=1e-12,
                    scalar2=1e12,
                    op0=mybir.AluOpType.max,
                    op1=mybir.AluOpType.min,
                )
            # One DMA for all outputs: dists[n, p] = s_all[p, n]
            nc.sync.dma_start(
                out=out_d[:].rearrange("n p -> p n"),
                in_=s_all[:],
            )
    nc.compile()
    return nc


def kernel(x, labels, centers):
    from concourse.bass_utils import run_bass_kernel_spmd

    x = np.ascontiguousarray(np.asarray(x, dtype=np.float32))
    centers = np.ascontiguousarray(np.asarray(centers, dtype=np.float32))
    labels = np.ascontiguousarray(
        np.asarray(labels).astype(np.int32).reshape(BATCH, 1)
    )

    impl = os.environ.get("CENTERLOSS_IMPL", "raw")
    if ("nc", impl) not in _CACHE:
        _CACHE[("nc", impl)] = _build_raw() if impl == "raw" else _build_bass()
    nc = _CACHE[("nc", impl)]

    core_ids = list(range(N_CORES))
    in_maps = [
        {
            "x": x[k * ROWS_PER_CORE : (k + 1) * ROWS_PER_CORE],
            "labels": labels[k * ROWS_PER_CORE : (k + 1) * ROWS_PER_CORE],
            "centers": centers,
        }
        for k in core_ids
    ]

    res = run_bass_kernel_spmd(nc, in_maps, core_ids)
    _CACHE["last_results"] = res

    dists = np.concatenate([res.results[k]["dists"].reshape(-1) for k in core_ids])
    # Reference clamps after masking: the label entry per row is clipped to
    # [1e-12, 1e12], and the B*(C-1) masked zeros each become 1e-12.
    dists = np.clip(dists, 1e-12, 1e12)
    total = dists.sum(dtype=np.float64) + BATCH * (NUM_CLASSES - 1) * 1e-12
    return np.float32(total / BATCH)
